# revision 17
# baseline (speedup 1.0000x reference)
"""Trainium2 Bass kernel for nn_DiversityLoss (cosine diversity loss).

Math: for each sample b with length L_b, the reference computes
    S = Xn @ Xn.T  (Xn = row-normalized, padding rows zeroed)
    sum_off[b] = sum(S) - L_b
    per_sample[b] = sum_off[b] / (L_b*(L_b-1))  if L_b > 1 else 0
    out = sum(per_sample) / count(L_b != 1)

Key identity: sum(S) over the valid block equals ||sum_t xn_t||^2, so the
device only needs, per sample, v_b = sum over valid rows of x_t/||x_t||
(a length-D vector). The O(T^2) Gram matrix is never materialized.

Sharding: valid rows are tiled into 128-row sample-aligned tiles; the tiles
are distributed evenly over the 8 cores (balanced by actual row count, per
the data-parallel hint but load-balanced over the ragged lengths). Each core
computes z[g] = sum_p r[p,g] * x[p,g,:] per tile g via the tensor engine
(r = reciprocal row norms). The host reduces the per-tile partial sums into
per-sample vectors and applies the closed-form scalar epilogue (the
"all-reduce of the scalar numerator" from the hint).
"""

import math
from contextlib import ExitStack

import numpy as np

import concourse.bass as bass
import concourse.bacc as bacc
import concourse.tile as tile
from concourse import mybir
from concourse.bass_utils import run_bass_kernel_spmd

N_CORES = 8
P = 128  # rows per tile == SBUF partitions
D = 64   # feature dim (hardcoded for this problem)
EPS_SS = 1e-16  # bias inside sqrt; matches reference's clamp(norm, 1e-8)

_NC_CACHE: dict[tuple[int, int], bass.Bass] = {}


def _build_nc(G: int, n_chunks: int) -> bass.Bass:
    """Bass kernel: xp [128, G*64] f32 -> z [64, G] f32.

    Column group g of xp holds one 128-row tile (partition p = row p of the
    tile). Output column g is sum_p x[p,g,:] / ||x[p,g,:]||.
    """
    nc = bacc.Bacc()
    f32 = mybir.dt.float32
    xp = nc.dram_tensor("xp", [P, G * D], f32, kind="ExternalInput")
    zo = nc.dram_tensor("z", [D, G], f32, kind="ExternalOutput")

    xp3 = xp[:].rearrange("p (g d) -> p g d", d=D)

    bounds = []
    base, rem = divmod(G, n_chunks)
    g0 = 0
    for i in range(n_chunks):
        cg = base + (1 if i < rem else 0)
        if cg == 0:
            continue
        bounds.append((g0, g0 + cg))
        g0 += cg

    with tile.TileContext(nc) as tc, ExitStack() as ctx:
        # bufs >= number of chunks: no SBUF slot reuse, so DMAs never carry
        # WAR waits on multiple reader engines (one sync-wait slot per instr).
        pool = ctx.enter_context(tc.tile_pool(name="main", bufs=max(2, len(bounds))))
        small = ctx.enter_context(tc.tile_pool(name="small", bufs=1))
        ppool = ctx.enter_context(tc.tile_pool(name="psum", bufs=1, space="PSUM"))

        eps_t = small.tile([P, 1], f32)
        nc.vector.memset(eps_t, EPS_SS)
        psum_z = ppool.tile([D, G], f32)
        scratch = ppool.tile([1, max(1, len(bounds))], f32)
        z_sb = small.tile([D, G], f32)

        for ci, (g0, g1) in enumerate(bounds):
            cg = g1 - g0
            x_t = pool.tile([P, cg, D], f32, tag="x")
            nc.sync.dma_start(out=x_t, in_=xp3[:, g0:g1, :])
            # Touch x_t on PE with a single-dep throwaway matmul. Every TPB
            # instruction has one sync-wait slot and fp32 Matmult cannot split
            # an extra wait onto its internal weight load, so the real matmuls
            # below must not need both the DMA wait and the DVE wait.
            nc.tensor.matmul(
                scratch[0:1, ci : ci + 1],
                lhsT=x_t[:, 0, 0:1],
                rhs=x_t[:, 0, 0:1],
                start=True,
                stop=True,
            )
            sq_t = pool.tile([P, cg, D], f32, tag="sq")
            nc.scalar.square(sq_t, x_t)
            ss_t = pool.tile([P, cg], f32, tag="ss")
            nc.vector.reduce_sum(ss_t, sq_t[:, :, :], axis=mybir.AxisListType.X)
            # norm = sqrt(ss + eps); r = 1/norm  (Rsqrt activation is banned)
            nc.scalar.activation(
                ss_t, ss_t, mybir.ActivationFunctionType.Sqrt, bias=eps_t, scale=1.0
            )
            nc.vector.reciprocal(ss_t, ss_t)
            for j in range(cg):
                nc.tensor.matmul(
                    psum_z[:, g0 + j : g0 + j + 1],
                    lhsT=x_t[:, j, :],
                    rhs=ss_t[:, j : j + 1],
                    start=True,
                    stop=True,
                )
        nc.vector.tensor_copy(z_sb, psum_z)
        nc.sync.dma_start(out=zo[:, :], in_=z_sb[:, :])
    nc.compile()
    return nc


def _chunk_bounds(G: int, n_chunks: int):
    """Chunk [0, G) with a deliberately small first chunk so the first
    DMA lands (transfer + completion receipt) as early as possible."""
    if n_chunks <= 1 or G <= n_chunks:
        return [(0, G)]
    if n_chunks == 2:
        return [(0, G // 2), (G // 2, G)]
    first = max(1, min(round(G * 0.3), G - (n_chunks - 1)))
    rest = G - first
    bounds = [(0, first)]
    base, rem = divmod(rest, n_chunks - 1)
    g0 = first
    for i in range(n_chunks - 1):
        cg = base + (1 if i < rem else 0)
        if cg == 0:
            continue
        bounds.append((g0, g0 + cg))
        g0 += cg
    return bounds


def _build_nc_raw(G: int, n_chunks: int) -> bass.Bass:
    """Raw-Bass (hand-semaphored) version: no TileContext, so none of its
    kernel-tail drain/sem-clear barrier. Every cross-engine dependency is an
    explicit standalone wait.

    Per chunk: DMA(x) -> ACT square(f32) -> DVE grouped reduce + reciprocal
    (1/ss, back-to-back on DVE) -> ACT sqrt -> r = sqrt(1/ss) in bf16 ->
    PE matmul into psum columns. bf16 copies of x for the PE are made on
    DVE (chunk 0) and ACT (later chunks) to balance the two engines.
    The last input DMA is issued from GPSIMD's SWDGE queue in parallel with
    the sync queue's issues.
    """
    nc = bacc.Bacc()
    f32 = mybir.dt.float32
    bf16 = mybir.dt.bfloat16
    xp = nc.dram_tensor("xp", [P, G * D], f32, kind="ExternalInput")
    zo = nc.dram_tensor("z", [D, G], f32, kind="ExternalOutput")
    bounds = _chunk_bounds(G, n_chunks)
    C = len(bounds)
    gp_dmas = [C - 1] if C > 1 else []   # chunks issued by gpsimd (SWDGE)
    sync_dmas = [c for c in range(C) if c not in gp_dmas]

    with ExitStack() as ctx:
        en = ctx.enter_context
        xall = en(nc.sbuf_tensor("xall", [P, G * D], f32))
        xbf = en(nc.sbuf_tensor("xbf", [P, G * D], bf16))
        sqall = en(nc.sbuf_tensor("sqall", [P, G * D], f32))
        ss = en(nc.sbuf_tensor("ss", [P, G], f32))
        iss = en(nc.sbuf_tensor("iss", [P, G], f32))
        rbf = en(nc.sbuf_tensor("rbf", [P, G], bf16))
        zsb = en(nc.sbuf_tensor("zsb", [D, G], f32))
        pz = en(nc.psum_tensor("pz", [D, G], f32))
        dma_sems = [en(nc.semaphore(f"dma_sem{i}")) for i in range(C)]
        out_sem = en(nc.semaphore("out_sem"))
        sq_sem = en(nc.semaphore("sq_sem"))      # ACT square done (per chunk)
        rd_sem = en(nc.semaphore("rd_sem"))      # DVE reduce done (per chunk)
        rr_sem = en(nc.semaphore("rr_sem"))      # DVE red+recip done
        xc_sem = en(nc.semaphore("xc_sem"))      # bf16 x copy done (per chunk)
        rb_sem = en(nc.semaphore("rb_sem"))      # ACT sqrt -> rbf done
        pe_sem = en(nc.semaphore("pe_sem"))
        cp_sem = en(nc.semaphore("cp_sem"))

        # engine that makes the bf16 x copy, per chunk
        cast_eng = ["dve"] + ["act"] * (C - 1)

        with nc.Block(no_gpsimd_drain=True) as block:

            @block.sync
            def _(sync):
                for ci in sync_dmas:
                    g0, g1 = bounds[ci]
                    sync.dma_start(
                        out=xall[:, g0 * D : g1 * D], in_=xp[:, g0 * D : g1 * D]
                    ).then_inc(dma_sems[ci], 16)
                sync.wait_ge(cp_sem, 1)
                sync.dma_start(out=zo[:, :], in_=zsb[:, :]).then_inc(out_sem, 16)

            @block.gpsimd
            def _(gpsimd):
                for ci in gp_dmas:
                    g0, g1 = bounds[ci]
                    gpsimd.dma_start(
                        out=xall[:, g0 * D : g1 * D], in_=xp[:, g0 * D : g1 * D]
                    ).then_inc(dma_sems[ci], 16)

            @block.scalar
            def _(scalar):
                def do_sq(ci):
                    g0, g1 = bounds[ci]
                    scalar.wait_ge(dma_sems[ci], 16)
                    scalar.activation(
                        sqall[:, g0 * D : g1 * D],
                        xall[:, g0 * D : g1 * D],
                        mybir.ActivationFunctionType.Square,
                    ).then_inc(sq_sem, 1)

                def do_sqrt(ci):
                    g0, g1 = bounds[ci]
                    scalar.wait_ge(rr_sem, ci + 1)
                    with nc.allow_low_precision(
                        reason="bf16 r for the PE weighted-sum; norms stay f32"
                    ):
                        scalar.activation(
                            rbf[:, g0:g1],
                            iss[:, g0:g1],
                            mybir.ActivationFunctionType.Sqrt,
                        ).then_inc(rb_sem, 1)

                def do_cast(ci):
                    g0, g1 = bounds[ci]
                    scalar.activation(
                        xbf[:, g0 * D : g1 * D],
                        xall[:, g0 * D : g1 * D],
                        mybir.ActivationFunctionType.Copy,
                    ).then_inc(xc_sem, 1)

                do_sq(0)
                for ci in range(1, C):
                    do_sq(ci)          # dma wait covers the cast input too
                    do_sqrt(ci - 1)
                    if cast_eng[ci] == "act":
                        do_cast(ci)
                do_sqrt(C - 1)

            @block.vector
            def _(vector):
                def do_cast(ci):
                    g0, g1 = bounds[ci]
                    vector.wait_ge(dma_sems[ci], 16)
                    vector.tensor_copy(
                        xbf[:, g0 * D : g1 * D], xall[:, g0 * D : g1 * D]
                    ).then_inc(xc_sem, 1)

                for ci, (g0, g1) in enumerate(bounds):
                    if cast_eng[ci] == "dve":
                        do_cast(ci)
                    vector.wait_ge(sq_sem, ci + 1)
                    vector.reduce_sum(
                        ss[:, g0:g1],
                        sqall[:, g0 * D : g1 * D].rearrange(
                            "p (g d) -> p g d", d=D
                        ),
                        axis=mybir.AxisListType.X,
                    ).then_inc(rd_sem, 1)
                    vector.wait_ge(rd_sem, ci + 1)
                    vector.reciprocal(iss[:, g0:g1], ss[:, g0:g1]).then_inc(
                        rr_sem, 1
                    )
                vector.wait_ge(pe_sem, C)
                vector.tensor_copy(zsb[:, :], pz[:, :]).then_inc(cp_sem, 1)

            @block.tensor
            def _(tensor):
                for ci, (g0, g1) in enumerate(bounds):
                    tensor.wait_ge(rb_sem, ci + 1)
                    tensor.wait_ge(xc_sem, ci + 1)
                    for g in range(g0, g1):
                        mm = tensor.matmul(
                            pz[:, g : g + 1],
                            lhsT=xbf[:, g * D : (g + 1) * D],
                            rhs=rbf[:, g : g + 1],
                            start=True,
                            stop=True,
                        )
                    mm.then_inc(pe_sem, 1)

    nc.compile()
    _dedup_act_loads(nc)
    return nc


def _dedup_act_loads(nc) -> None:
    """Bacc inserts one ACT table load per activation family (Square and
    Sqrt live in different default sets), and the second ~1.3us load lands
    mid-pipeline right before the first Sqrt. One set (sqrt_and_friends)
    contains both functions, so retarget the first load and drop the rest."""
    from concourse.hw_specs import get_activation_tables

    sqrt_set_id = list(get_activation_tables(nc.m.arch).keys()).index(
        "sqrt_and_friends"
    )
    seen = False
    for func in nc.m.functions:
        for blk in func.blocks:
            insts = blk.instructions
            keep = []
            changed = False
            for inst in insts:
                if isinstance(inst, mybir.InstLoadActFuncSet):
                    if not seen:
                        inst.act_func_set_id = sqrt_set_id
                        seen = True
                        keep.append(inst)
                    else:
                        changed = True
                        continue
                else:
                    keep.append(inst)
            if changed:
                blk.instructions = keep


def _get_nc(G: int, n_chunks: int) -> bass.Bass:
    key = (G, n_chunks)
    if key not in _NC_CACHE:
        _NC_CACHE[key] = _build_nc_raw(G, n_chunks)
    return _NC_CACHE[key]


def _pack_inputs(target: np.ndarray, lens: np.ndarray):
    """Tile valid rows into 128-row sample-aligned tiles, balance over cores,
    and lay each core's tiles out partition-major ([128, G*64])."""
    B, T, Dd = target.shape
    assert Dd == D
    tiles = []  # (sample, t0, nrows)
    for b in range(B):
        L = int(lens[b])
        for t0 in range(0, L, P):
            tiles.append((b, t0, min(P, L - t0)))
    NT = len(tiles)
    G = max(1, math.ceil(NT / N_CORES))
    xps, gmaps, pads = [], [], []
    for c in range(N_CORES):
        sub = tiles[c * G : (c + 1) * G]
        # Padding rows are e0 = (1,0,...,0): unit norm, so the kernel (which
        # computes r = sqrt(1/ss) with NO epsilon) sees ss=1 and each pad row
        # contributes exactly e0 to its group sum; the host subtracts the
        # known pad counts afterwards. Avoids inf/NaN from all-zero rows.
        buf = np.zeros((G, P, D), dtype=np.float32)
        buf[:, :, 0] = 1.0
        gmap = np.full((G,), -1, dtype=np.int64)
        pad = np.full((G,), P, dtype=np.int64)
        for g, (b, t0, rows) in enumerate(sub):
            buf[g, :rows, :] = target[b, t0 : t0 + rows, :]
            gmap[g] = b
            pad[g] = P - rows
        xps.append(np.ascontiguousarray(buf.transpose(1, 0, 2)).reshape(P, G * D))
        gmaps.append(gmap)
        pads.append(pad)
    return xps, gmaps, pads, G


def kernel(target: np.ndarray, target_len: np.ndarray, _run_kwargs=None):
    target = np.asarray(target, dtype=np.float32)
    lens = np.asarray(target_len)
    B = target.shape[0]

    xps, gmaps, pads, G = _pack_inputs(target, lens)
    n_chunks = min(2, G)
    nc = _get_nc(G, n_chunks)

    in_maps = [{"xp": xps[c]} for c in range(N_CORES)]
    res = run_bass_kernel_spmd(
        nc, in_maps, core_ids=list(range(N_CORES)), **(_run_kwargs or {})
    )
    if _run_kwargs is not None:
        _run_kwargs["_last_result"] = res

    # host epilogue: combine per-tile partials into per-sample vectors
    V = np.zeros((B, D), dtype=np.float64)
    for c in range(N_CORES):
        z = np.asarray(res.results[c]["z"], dtype=np.float64).T  # [G, 64]
        z[:, 0] -= pads[c]  # remove the e0 padding-row contributions
        gm = gmaps[c]
        for b in range(B):
            sel = gm == b
            if sel.any():
                V[b] += z[sel].sum(axis=0)

    lens_f = lens.astype(np.float64)
    ssb = (V * V).sum(axis=1)  # ||v_b||^2 == sum(S_b)
    sum_off = ssb - lens_f
    pair = np.where(lens_f > 1, lens_f * (lens_f - 1.0), 1.0)
    per_sample = np.where(lens_f > 1, sum_off / pair, 0.0)
    denom = float((lens_f != 1).sum())
    return np.asarray(per_sample.sum() / denom, dtype=np.float32)


# revision 21
# speedup vs baseline: 1.0069x; 1.0069x over previous
"""Trainium2 Bass kernel for nn_DiversityLoss (cosine diversity loss).

Math: for each sample b with length L_b, the reference computes
    S = Xn @ Xn.T  (Xn = row-normalized, padding rows zeroed)
    sum_off[b] = sum(S) - L_b
    per_sample[b] = sum_off[b] / (L_b*(L_b-1))  if L_b > 1 else 0
    out = sum(per_sample) / count(L_b != 1)

Key identity: sum(S) over the valid block equals ||sum_t xn_t||^2, so the
device only needs, per sample, v_b = sum over valid rows of x_t/||x_t||
(a length-D vector). The O(T^2) Gram matrix is never materialized.

Sharding: valid rows are tiled into 128-row sample-aligned tiles; the tiles
are distributed evenly over the 8 cores (balanced by actual row count, per
the data-parallel hint but load-balanced over the ragged lengths). Each core
computes z[g] = sum_p r[p,g] * x[p,g,:] per tile g via the tensor engine
(r = reciprocal row norms). The host reduces the per-tile partial sums into
per-sample vectors and applies the closed-form scalar epilogue (the
"all-reduce of the scalar numerator" from the hint).
"""

import math
from contextlib import ExitStack

import numpy as np

import concourse.bass as bass
import concourse.bacc as bacc
from concourse import mybir
from concourse.bass_utils import run_bass_kernel_spmd

N_CORES = 8
P = 128  # rows per tile == SBUF partitions
D = 64   # feature dim (hardcoded for this problem)
EPS_SS = 1e-16  # bias inside sqrt; matches reference's clamp(norm, 1e-8)

_NC_CACHE: dict[tuple[int, int], bass.Bass] = {}


def _chunk_bounds(G: int, n_chunks: int):
    """Chunk [0, G) with a deliberately small first chunk so the first
    DMA lands (transfer + completion receipt) as early as possible."""
    if n_chunks <= 1 or G <= n_chunks:
        return [(0, G)]
    if n_chunks == 2:
        return [(0, G // 2), (G // 2, G)]
    first = max(1, min(round(G * 0.18), G - (n_chunks - 1)))
    rest = G - first
    bounds = [(0, first)]
    base, rem = divmod(rest, n_chunks - 1)
    g0 = first
    for i in range(n_chunks - 1):
        cg = base + (1 if i < rem else 0)
        if cg == 0:
            continue
        bounds.append((g0, g0 + cg))
        g0 += cg
    return bounds


def _build_nc_raw(G: int, n_chunks: int) -> bass.Bass:
    """Raw-Bass (hand-semaphored) version: no TileContext, so none of its
    kernel-tail drain/sem-clear barrier. Every cross-engine dependency is an
    explicit standalone wait.

    Per chunk: DMA(x) -> ACT square(f32) -> DVE grouped reduce + reciprocal
    (1/ss, back-to-back on DVE) -> ACT sqrt -> r = sqrt(1/ss) in bf16 ->
    PE matmul into psum columns. bf16 copies of x for the PE are made on
    DVE (chunk 0) and ACT (later chunks) to balance the two engines.
    The last input DMA is issued from GPSIMD's SWDGE queue in parallel with
    the sync queue's issues.
    """
    nc = bacc.Bacc()
    f32 = mybir.dt.float32
    bf16 = mybir.dt.bfloat16
    xp = nc.dram_tensor("xp", [P, G * D], f32, kind="ExternalInput")
    zo = nc.dram_tensor("z", [D, G], f32, kind="ExternalOutput")
    bounds = _chunk_bounds(G, n_chunks)
    C = len(bounds)
    gp_dmas = [C - 1] if C > 1 else []   # chunks issued by gpsimd (SWDGE)
    sync_dmas = [c for c in range(C) if c not in gp_dmas]

    with ExitStack() as ctx:
        en = ctx.enter_context
        xall = en(nc.sbuf_tensor("xall", [P, G * D], f32))
        xbf = en(nc.sbuf_tensor("xbf", [P, G * D], bf16))
        sqall = en(nc.sbuf_tensor("sqall", [P, G * D], f32))
        ss = en(nc.sbuf_tensor("ss", [P, G], f32))
        iss = en(nc.sbuf_tensor("iss", [P, G], f32))
        rbf = en(nc.sbuf_tensor("rbf", [P, G], bf16))
        zsb = en(nc.sbuf_tensor("zsb", [D, G], f32))
        pz = en(nc.psum_tensor("pz", [D, G], f32))
        dma_sems = [en(nc.semaphore(f"dma_sem{i}")) for i in range(C)]
        out_sem = en(nc.semaphore("out_sem"))
        sq_sem = en(nc.semaphore("sq_sem"))      # ACT square done (per chunk)
        rd_sem = en(nc.semaphore("rd_sem"))      # DVE reduce done (per chunk)
        rr_sem = en(nc.semaphore("rr_sem"))      # DVE red+recip done
        # one cast sem per chunk: DVE and ACT both produce casts, and a
        # shared counting sem would let one engine's increment satisfy a
        # wait for the other engine's (unfinished) cast
        xc_sems = [en(nc.semaphore(f"xc_sem{i}")) for i in range(C)]
        rb_sem = en(nc.semaphore("rb_sem"))      # ACT sqrt -> rbf done
        pe_sem = en(nc.semaphore("pe_sem"))
        cp_sem = en(nc.semaphore("cp_sem"))

        # engine that makes the bf16 x copy, per chunk
        cast_eng = ["dve"] + ["act"] * (C - 1)

        with nc.Block(no_gpsimd_drain=True) as block:

            @block.sync
            def _(sync):
                for ci in sync_dmas:
                    g0, g1 = bounds[ci]
                    sync.dma_start(
                        out=xall[:, g0 * D : g1 * D], in_=xp[:, g0 * D : g1 * D]
                    ).then_inc(dma_sems[ci], 16)
                sync.wait_ge(cp_sem, 1)
                sync.dma_start(out=zo[:, :], in_=zsb[:, :]).then_inc(out_sem, 16)

            @block.gpsimd
            def _(gpsimd):
                for ci in gp_dmas:
                    g0, g1 = bounds[ci]
                    gpsimd.dma_start(
                        out=xall[:, g0 * D : g1 * D], in_=xp[:, g0 * D : g1 * D]
                    ).then_inc(dma_sems[ci], 16)

            @block.scalar
            def _(scalar):
                def do_sq(ci):
                    g0, g1 = bounds[ci]
                    scalar.wait_ge(dma_sems[ci], 16)
                    scalar.activation(
                        sqall[:, g0 * D : g1 * D],
                        xall[:, g0 * D : g1 * D],
                        mybir.ActivationFunctionType.Square,
                    ).then_inc(sq_sem, 1)

                def do_sqrt(ci):
                    g0, g1 = bounds[ci]
                    scalar.wait_ge(rr_sem, ci + 1)
                    with nc.allow_low_precision(
                        reason="bf16 r for the PE weighted-sum; norms stay f32"
                    ):
                        scalar.activation(
                            rbf[:, g0:g1],
                            iss[:, g0:g1],
                            mybir.ActivationFunctionType.Sqrt,
                        ).then_inc(rb_sem, 1)

                def do_cast(ci):
                    g0, g1 = bounds[ci]
                    scalar.activation(
                        xbf[:, g0 * D : g1 * D],
                        xall[:, g0 * D : g1 * D],
                        mybir.ActivationFunctionType.Copy,
                    ).then_inc(xc_sems[ci], 1)

                do_sq(0)
                for ci in range(1, C):
                    do_sq(ci)          # dma wait covers the cast input too
                    do_sqrt(ci - 1)
                    if cast_eng[ci] == "act":
                        do_cast(ci)
                do_sqrt(C - 1)

            @block.vector
            def _(vector):
                def do_cast(ci):
                    g0, g1 = bounds[ci]
                    vector.wait_ge(dma_sems[ci], 16)
                    vector.tensor_copy(
                        xbf[:, g0 * D : g1 * D], xall[:, g0 * D : g1 * D]
                    ).then_inc(xc_sems[ci], 1)

                for ci, (g0, g1) in enumerate(bounds):
                    if cast_eng[ci] == "dve":
                        do_cast(ci)
                    vector.wait_ge(sq_sem, ci + 1)
                    vector.reduce_sum(
                        ss[:, g0:g1],
                        sqall[:, g0 * D : g1 * D].rearrange(
                            "p (g d) -> p g d", d=D
                        ),
                        axis=mybir.AxisListType.X,
                    ).then_inc(rd_sem, 1)
                    vector.wait_ge(rd_sem, ci + 1)
                    vector.reciprocal(iss[:, g0:g1], ss[:, g0:g1]).then_inc(
                        rr_sem, 1
                    )
                vector.wait_ge(pe_sem, C)
                vector.tensor_copy(zsb[:, :], pz[:, :]).then_inc(cp_sem, 1)

            @block.tensor
            def _(tensor):
                for ci, (g0, g1) in enumerate(bounds):
                    tensor.wait_ge(rb_sem, ci + 1)
                    tensor.wait_ge(xc_sems[ci], 1)
                    for g in range(g0, g1):
                        mm = tensor.matmul(
                            pz[:, g : g + 1],
                            lhsT=xbf[:, g * D : (g + 1) * D],
                            rhs=rbf[:, g : g + 1],
                            start=True,
                            stop=True,
                        )
                    mm.then_inc(pe_sem, 1)

    nc.compile()
    _dedup_act_loads(nc)
    return nc


def _dedup_act_loads(nc) -> None:
    """Bacc inserts one ACT table load per activation family (Square and
    Sqrt live in different default sets), and the second ~1.3us load lands
    mid-pipeline right before the first Sqrt. One set (sqrt_and_friends)
    contains both functions, so retarget the first load and drop the rest."""
    from concourse.hw_specs import get_activation_tables

    sqrt_set_id = list(get_activation_tables(nc.m.arch).keys()).index(
        "sqrt_and_friends"
    )
    seen = False
    for func in nc.m.functions:
        for blk in func.blocks:
            insts = blk.instructions
            keep = []
            changed = False
            for inst in insts:
                if isinstance(inst, mybir.InstLoadActFuncSet):
                    if not seen:
                        inst.act_func_set_id = sqrt_set_id
                        seen = True
                        keep.append(inst)
                    else:
                        changed = True
                        continue
                else:
                    keep.append(inst)
            if changed:
                blk.instructions = keep


def _get_nc(G: int, n_chunks: int) -> bass.Bass:
    key = (G, n_chunks)
    if key not in _NC_CACHE:
        _NC_CACHE[key] = _build_nc_raw(G, n_chunks)
    return _NC_CACHE[key]


def _pack_inputs(target: np.ndarray, lens: np.ndarray):
    """Tile valid rows into 128-row sample-aligned tiles, balance over cores,
    and lay each core's tiles out partition-major ([128, G*64])."""
    B, T, Dd = target.shape
    assert Dd == D
    tiles = []  # (sample, t0, nrows)
    for b in range(B):
        L = int(lens[b])
        for t0 in range(0, L, P):
            tiles.append((b, t0, min(P, L - t0)))
    NT = len(tiles)
    G = max(1, math.ceil(NT / N_CORES))
    xps, gmaps, pads = [], [], []
    for c in range(N_CORES):
        sub = tiles[c * G : (c + 1) * G]
        # Padding rows are e0 = (1,0,...,0): unit norm, so the kernel (which
        # computes r = sqrt(1/ss) with NO epsilon) sees ss=1 and each pad row
        # contributes exactly e0 to its group sum; the host subtracts the
        # known pad counts afterwards. Avoids inf/NaN from all-zero rows.
        buf = np.zeros((G, P, D), dtype=np.float32)
        buf[:, :, 0] = 1.0
        gmap = np.full((G,), -1, dtype=np.int64)
        pad = np.full((G,), P, dtype=np.int64)
        for g, (b, t0, rows) in enumerate(sub):
            buf[g, :rows, :] = target[b, t0 : t0 + rows, :]
            gmap[g] = b
            pad[g] = P - rows
        xps.append(np.ascontiguousarray(buf.transpose(1, 0, 2)).reshape(P, G * D))
        gmaps.append(gmap)
        pads.append(pad)
    return xps, gmaps, pads, G


def kernel(target: np.ndarray, target_len: np.ndarray, _run_kwargs=None):
    target = np.asarray(target, dtype=np.float32)
    lens = np.asarray(target_len)
    B = target.shape[0]

    xps, gmaps, pads, G = _pack_inputs(target, lens)
    n_chunks = min(2, G)
    nc = _get_nc(G, n_chunks)

    in_maps = [{"xp": xps[c]} for c in range(N_CORES)]
    res = run_bass_kernel_spmd(
        nc, in_maps, core_ids=list(range(N_CORES)), **(_run_kwargs or {})
    )
    if _run_kwargs is not None:
        _run_kwargs["_last_result"] = res

    # host epilogue: combine per-tile partials into per-sample vectors
    V = np.zeros((B, D), dtype=np.float64)
    for c in range(N_CORES):
        z = np.asarray(res.results[c]["z"], dtype=np.float64).T  # [G, 64]
        z[:, 0] -= pads[c]  # remove the e0 padding-row contributions
        gm = gmaps[c]
        for b in range(B):
            sel = gm == b
            if sel.any():
                V[b] += z[sel].sum(axis=0)

    lens_f = lens.astype(np.float64)
    ssb = (V * V).sum(axis=1)  # ||v_b||^2 == sum(S_b)
    sum_off = ssb - lens_f
    pair = np.where(lens_f > 1, lens_f * (lens_f - 1.0), 1.0)
    per_sample = np.where(lens_f > 1, sum_off / pair, 0.0)
    denom = float((lens_f != 1).sum())
    return np.asarray(per_sample.sum() / denom, dtype=np.float32)


# revision 22
# speedup vs baseline: 1.0218x; 1.0148x over previous
"""Trainium2 Bass kernel for nn_DiversityLoss (cosine diversity loss).

Math: for each sample b with length L_b, the reference computes
    S = Xn @ Xn.T  (Xn = row-normalized, padding rows zeroed)
    sum_off[b] = sum(S) - L_b
    per_sample[b] = sum_off[b] / (L_b*(L_b-1))  if L_b > 1 else 0
    out = sum(per_sample) / count(L_b != 1)

Key identity: sum(S) over the valid block equals ||sum_t xn_t||^2, so the
device only needs, per sample, v_b = sum over valid rows of x_t/||x_t||
(a length-D vector). The O(T^2) Gram matrix is never materialized.

Sharding: valid rows are tiled into 128-row sample-aligned tiles; the tiles
are distributed evenly over the 8 cores (balanced by actual row count, per
the data-parallel hint but load-balanced over the ragged lengths). Each core
computes z[g] = sum_p r[p,g] * x[p,g,:] per tile g via the tensor engine
(r = reciprocal row norms). The host reduces the per-tile partial sums into
per-sample vectors and applies the closed-form scalar epilogue (the
"all-reduce of the scalar numerator" from the hint).
"""

import math
from contextlib import ExitStack

import numpy as np

import concourse.bass as bass
import concourse.bacc as bacc
from concourse import mybir
from concourse.bass_utils import run_bass_kernel_spmd

N_CORES = 8
P = 128  # rows per tile == SBUF partitions
D = 64   # feature dim (hardcoded for this problem)

_NC_CACHE: dict[tuple[int, int], bass.Bass] = {}


def _chunk_bounds(G: int, n_chunks: int):
    """Chunk [0, G) with a deliberately small first chunk so the first
    DMA lands (transfer + completion receipt) as early as possible."""
    if n_chunks <= 1 or G <= n_chunks:
        return [(0, G)]
    if n_chunks == 2:
        return [(0, G // 2), (G // 2, G)]
    first = max(1, min(round(G * 0.18), G - (n_chunks - 1)))
    rest = G - first
    bounds = [(0, first)]
    base, rem = divmod(rest, n_chunks - 1)
    g0 = first
    for i in range(n_chunks - 1):
        cg = base + (1 if i < rem else 0)
        if cg == 0:
            continue
        bounds.append((g0, g0 + cg))
        g0 += cg
    return bounds


def _build_nc_raw(G: int, n_chunks: int) -> bass.Bass:
    """Raw-Bass (hand-semaphored) version: no TileContext, so none of its
    kernel-tail drain/sem-clear barrier. Every cross-engine dependency is an
    explicit standalone wait.

    Per chunk: DMA(x) -> ACT square(f32) -> DVE grouped reduce + reciprocal
    (1/ss, back-to-back on DVE) -> ACT sqrt -> r = sqrt(1/ss) in bf16 ->
    PE matmul into psum columns. bf16 copies of x for the PE are made on
    DVE (chunk 0) and ACT (later chunks) to balance the two engines.
    The last input DMA is issued from GPSIMD's SWDGE queue in parallel with
    the sync queue's issues.
    """
    nc = bacc.Bacc()
    f32 = mybir.dt.float32
    bf16 = mybir.dt.bfloat16
    xp = nc.dram_tensor("xp", [P, G * D], f32, kind="ExternalInput")
    zo = nc.dram_tensor("z", [D, G], f32, kind="ExternalOutput")
    bounds = _chunk_bounds(G, n_chunks)
    C = len(bounds)
    gp_dmas = [C - 1] if C > 1 else []   # chunks issued by gpsimd (SWDGE)
    sync_dmas = [c for c in range(C) if c not in gp_dmas]

    with ExitStack() as ctx:
        en = ctx.enter_context
        xall = en(nc.sbuf_tensor("xall", [P, G * D], f32))
        xbf = en(nc.sbuf_tensor("xbf", [P, G * D], bf16))
        sqall = en(nc.sbuf_tensor("sqall", [P, G * D], f32))
        ss = en(nc.sbuf_tensor("ss", [P, G], f32))
        iss = en(nc.sbuf_tensor("iss", [P, G], f32))
        rbf = en(nc.sbuf_tensor("rbf", [P, G], bf16))
        zsb = en(nc.sbuf_tensor("zsb", [D, G], f32))
        pz = en(nc.psum_tensor("pz", [D, G], f32))
        dma_sems = [en(nc.semaphore(f"dma_sem{i}")) for i in range(C)]
        out_sem = en(nc.semaphore("out_sem"))
        sq_sem = en(nc.semaphore("sq_sem"))      # ACT square done (per chunk)
        rd_sem = en(nc.semaphore("rd_sem"))      # DVE reduce done (per chunk)
        rr_sem = en(nc.semaphore("rr_sem"))      # DVE red+recip done
        # one cast sem per chunk: DVE and ACT both produce casts, and a
        # shared counting sem would let one engine's increment satisfy a
        # wait for the other engine's (unfinished) cast
        xc_sems = [en(nc.semaphore(f"xc_sem{i}")) for i in range(C)]
        rb_sem = en(nc.semaphore("rb_sem"))      # ACT sqrt -> rbf done
        pe_sem = en(nc.semaphore("pe_sem"))
        cp_sem = en(nc.semaphore("cp_sem"))

        # engine that makes the bf16 x copy, per chunk
        cast_eng = ["dve"] + ["act"] * (C - 1)

        with nc.Block(no_gpsimd_drain=True) as block:

            @block.sync
            def _(sync):
                for ci in sync_dmas:
                    g0, g1 = bounds[ci]
                    sync.dma_start(
                        out=xall[:, g0 * D : g1 * D], in_=xp[:, g0 * D : g1 * D]
                    ).then_inc(dma_sems[ci], 16)
                sync.wait_ge(cp_sem, 1)
                sync.dma_start(out=zo[:, :], in_=zsb[:, :]).then_inc(out_sem, 16)

            @block.gpsimd
            def _(gpsimd):
                for ci in gp_dmas:
                    g0, g1 = bounds[ci]
                    gpsimd.dma_start(
                        out=xall[:, g0 * D : g1 * D], in_=xp[:, g0 * D : g1 * D]
                    ).then_inc(dma_sems[ci], 16)

            @block.scalar
            def _(scalar):
                def do_sq(ci):
                    g0, g1 = bounds[ci]
                    scalar.wait_ge(dma_sems[ci], 16)
                    scalar.activation(
                        sqall[:, g0 * D : g1 * D],
                        xall[:, g0 * D : g1 * D],
                        mybir.ActivationFunctionType.Square,
                    ).then_inc(sq_sem, 1)

                def do_sqrt(ci):
                    g0, g1 = bounds[ci]
                    scalar.wait_ge(rr_sem, ci + 1)
                    with nc.allow_low_precision(
                        reason="bf16 r for the PE weighted-sum; norms stay f32"
                    ):
                        scalar.activation(
                            rbf[:, g0:g1],
                            iss[:, g0:g1],
                            mybir.ActivationFunctionType.Sqrt,
                        ).then_inc(rb_sem, 1)

                def do_cast(ci):
                    g0, g1 = bounds[ci]
                    scalar.activation(
                        xbf[:, g0 * D : g1 * D],
                        xall[:, g0 * D : g1 * D],
                        mybir.ActivationFunctionType.Copy,
                    ).then_inc(xc_sems[ci], 1)

                do_sq(0)
                for ci in range(1, C):
                    do_sq(ci)          # dma wait covers the cast input too
                    do_sqrt(ci - 1)
                    if cast_eng[ci] == "act":
                        do_cast(ci)
                do_sqrt(C - 1)

            @block.vector
            def _(vector):
                def do_cast(ci):
                    g0, g1 = bounds[ci]
                    vector.wait_ge(dma_sems[ci], 16)
                    vector.tensor_copy(
                        xbf[:, g0 * D : g1 * D], xall[:, g0 * D : g1 * D]
                    ).then_inc(xc_sems[ci], 1)

                for ci, (g0, g1) in enumerate(bounds):
                    if cast_eng[ci] == "dve":
                        do_cast(ci)
                    vector.wait_ge(sq_sem, ci + 1)
                    vector.reduce_sum(
                        ss[:, g0:g1],
                        sqall[:, g0 * D : g1 * D].rearrange(
                            "p (g d) -> p g d", d=D
                        ),
                        axis=mybir.AxisListType.X,
                    ).then_inc(rd_sem, 1)
                    vector.wait_ge(rd_sem, ci + 1)
                    vector.reciprocal(iss[:, g0:g1], ss[:, g0:g1]).then_inc(
                        rr_sem, 1
                    )
                vector.wait_ge(pe_sem, C)
                vector.tensor_copy(zsb[:, :], pz[:, :]).then_inc(cp_sem, 1)

            @block.tensor
            def _(tensor):
                for ci, (g0, g1) in enumerate(bounds):
                    tensor.wait_ge(rb_sem, ci + 1)
                    tensor.wait_ge(xc_sems[ci], 1)
                    for g in range(g0, g1):
                        mm = tensor.matmul(
                            pz[:, g : g + 1],
                            lhsT=xbf[:, g * D : (g + 1) * D],
                            rhs=rbf[:, g : g + 1],
                            start=True,
                            stop=True,
                        )
                    mm.then_inc(pe_sem, 1)

    nc.compile()
    _dedup_act_loads(nc)
    return nc


def _dedup_act_loads(nc) -> None:
    """Bacc inserts one ACT table load per activation family (Square and
    Sqrt live in different default sets), and the second ~1.3us load lands
    mid-pipeline right before the first Sqrt. One set (sqrt_and_friends)
    contains both functions, so retarget the first load and drop the rest."""
    from concourse.hw_specs import get_activation_tables

    sqrt_set_id = list(get_activation_tables(nc.m.arch).keys()).index(
        "sqrt_and_friends"
    )
    seen = False
    for func in nc.m.functions:
        for blk in func.blocks:
            insts = blk.instructions
            keep = []
            changed = False
            for inst in insts:
                if isinstance(inst, mybir.InstLoadActFuncSet):
                    if not seen:
                        inst.act_func_set_id = sqrt_set_id
                        seen = True
                        keep.append(inst)
                    else:
                        changed = True
                        continue
                else:
                    keep.append(inst)
            if changed:
                blk.instructions = keep


def _get_nc(G: int, n_chunks: int) -> bass.Bass:
    key = (G, n_chunks)
    if key not in _NC_CACHE:
        _NC_CACHE[key] = _build_nc_raw(G, n_chunks)
    return _NC_CACHE[key]


def _pack_inputs(target: np.ndarray, lens: np.ndarray):
    """Tile valid rows into 128-row sample-aligned tiles, balance over cores,
    and lay each core's tiles out partition-major ([128, G*64])."""
    B, T, Dd = target.shape
    assert Dd == D
    tiles = []  # (sample, t0, nrows)
    for b in range(B):
        L = int(lens[b])
        for t0 in range(0, L, P):
            tiles.append((b, t0, min(P, L - t0)))
    NT = len(tiles)
    G = max(1, math.ceil(NT / N_CORES))
    xps, gmaps, pads = [], [], []
    for c in range(N_CORES):
        sub = tiles[c * G : (c + 1) * G]
        # Padding rows are e0 = (1,0,...,0): unit norm, so the kernel (which
        # computes r = sqrt(1/ss) with NO epsilon) sees ss=1 and each pad row
        # contributes exactly e0 to its group sum; the host subtracts the
        # known pad counts afterwards. Avoids inf/NaN from all-zero rows.
        buf = np.zeros((G, P, D), dtype=np.float32)
        buf[:, :, 0] = 1.0
        gmap = np.full((G,), -1, dtype=np.int64)
        pad = np.full((G,), P, dtype=np.int64)
        for g, (b, t0, rows) in enumerate(sub):
            buf[g, :rows, :] = target[b, t0 : t0 + rows, :]
            gmap[g] = b
            pad[g] = P - rows
        xps.append(np.ascontiguousarray(buf.transpose(1, 0, 2)).reshape(P, G * D))
        gmaps.append(gmap)
        pads.append(pad)
    return xps, gmaps, pads, G


def kernel(target: np.ndarray, target_len: np.ndarray, _run_kwargs=None):
    target = np.asarray(target, dtype=np.float32)
    lens = np.asarray(target_len)
    B = target.shape[0]

    xps, gmaps, pads, G = _pack_inputs(target, lens)
    n_chunks = min(2, G)
    nc = _get_nc(G, n_chunks)

    in_maps = [{"xp": xps[c]} for c in range(N_CORES)]
    res = run_bass_kernel_spmd(
        nc, in_maps, core_ids=list(range(N_CORES)), **(_run_kwargs or {})
    )
    if _run_kwargs is not None:
        _run_kwargs["_last_result"] = res

    # host epilogue: combine per-tile partials into per-sample vectors
    V = np.zeros((B, D), dtype=np.float64)
    for c in range(N_CORES):
        z = np.asarray(res.results[c]["z"], dtype=np.float64).T  # [G, 64]
        z[:, 0] -= pads[c]  # remove the e0 padding-row contributions
        gm = gmaps[c]
        for b in range(B):
            sel = gm == b
            if sel.any():
                V[b] += z[sel].sum(axis=0)

    lens_f = lens.astype(np.float64)
    ssb = (V * V).sum(axis=1)  # ||v_b||^2 == sum(S_b)
    sum_off = ssb - lens_f
    pair = np.where(lens_f > 1, lens_f * (lens_f - 1.0), 1.0)
    per_sample = np.where(lens_f > 1, sum_off / pair, 0.0)
    denom = float((lens_f != 1).sum())
    return np.asarray(per_sample.sum() / denom, dtype=np.float32)
